# revision 3
# baseline (speedup 1.0000x reference)
"""KV page-cache scatter update on 8 Trainium2 NeuronCores.

Semantics (matches the reference):
    kv_ev = interleave(new_k, new_v)          # [T, 2H, D], head axis k0,v0,k1,v1,...
    for i in range(K):
        kv_pages[t_pages[i], t_slots[i]] = kv_ev[i]
    return kv_pages

Sharding: kv_pages is split along the page axis across the 8 cores
(256 pages each).  The host partitions the valid tokens by destination
page group and hands each core a compacted, interleaved update block plus
flat destination row indices.  Each core:
  1. copies its 33.5MB page shard input -> output with large DRAM->DRAM DMAs
  2. loads its update rows into SBUF and scatters them into the output with
     an indirect (SWDGE) DMA using the destination row indices.
Destinations are unique (page,slot) pairs, so padding duplicates the last
valid row (identical concurrent writes are benign).
"""

import numpy as np

from concourse import bacc, bass, mybir, tile
from concourse.bass_utils import run_bass_kernel_spmd

# Problem geometry (hardcoded per contract).
P, S, HH, D = 2048, 16, 16, 128   # pages, slots/page, 2*kv_heads, head_dim
T = 2048                          # new tokens
NCORES = 8
PC = P // NCORES                  # pages per core
RC = PC * S                       # flat rows per core (4096)
RD = HH * D                       # row width in f32 (2048 = 8KB)

_PROGRAM_CACHE: dict[int, object] = {}
_LAST_IN_MAPS: list | None = None  # stashed for test.py's traced re-run


def _build_program(n_pad: int):
    """Bass program: copy kv shard in->out, then scatter n_pad update rows."""
    nc = bacc.Bacc("TRN2", target_bir_lowering=False, debug=False)

    kv_in = nc.dram_tensor("kv_in", [RC, RD], mybir.dt.float32, kind="ExternalInput")
    upd = nc.dram_tensor("upd", [n_pad, RD], mybir.dt.float32, kind="ExternalInput")
    dest = nc.dram_tensor("dest", [n_pad, 1], mybir.dt.int32, kind="ExternalInput")
    kv_out = nc.dram_tensor("kv_out", [RC, RD], mybir.dt.float32, kind="ExternalOutput")

    n_blocks = n_pad // 128
    total_elems = RC * RD
    n_chunks = 8
    chunk_elems = total_elems // n_chunks
    # inner descriptor rows of 8192 f32 (32KB), well under the 64KB AP limit
    inner = 8192
    chunk_rows = chunk_elems // inner

    with tile.TileContext(nc) as tc:
        with tc.tile_pool(name="sbuf", bufs=2) as pool:
            # bulk copy: 8 chunks x 4MB, DRAM->DRAM
            for i in range(n_chunks):
                off = i * chunk_elems
                src = bass.AP(kv_in, off, [[inner, chunk_rows], [1, inner]])
                dst = bass.AP(kv_out, off, [[inner, chunk_rows], [1, inner]])
                nc.sync.dma_start(out=dst, in_=src)

            # scatter: blocks of 128 rows through SBUF
            for b in range(n_blocks):
                utile = pool.tile([128, RD], mybir.dt.float32)
                dtile = pool.tile([128, 1], mybir.dt.int32)
                nc.sync.dma_start(out=utile[:], in_=upd[b * 128:(b + 1) * 128, :])
                nc.sync.dma_start(out=dtile[:], in_=dest[b * 128:(b + 1) * 128, :])
                nc.gpsimd.indirect_dma_start(
                    out=kv_out[:],
                    out_offset=bass.IndirectOffsetOnAxis(ap=dtile[:, :1], axis=0),
                    in_=utile[:],
                    in_offset=None,
                )

    nc.compile()
    return nc


def kernel(kv_pages, t_pages, t_slots, new_k, new_v, K):
    kv_pages = np.asarray(kv_pages)
    t_pages = np.asarray(t_pages)
    t_slots = np.asarray(t_slots)
    new_k = np.asarray(new_k)
    new_v = np.asarray(new_v)
    k_valid = int(np.asarray(K))

    out_dtype = kv_pages.dtype
    Tn, Hn, Dn = new_k.shape

    # interleave K/V along the head axis: [T, 2H, D] -> flat [T, RD]
    kv_ev = np.empty((Tn, 2 * Hn, Dn), dtype=out_dtype)
    kv_ev[:, 0::2, :] = new_k
    kv_ev[:, 1::2, :] = new_v
    kv_ev = kv_ev.reshape(Tn, 2 * Hn * Dn)

    tp = t_pages[:k_valid].astype(np.int64)
    ts = t_slots[:k_valid].astype(np.int64)
    core_of = tp // PC

    sels = [np.nonzero(core_of == c)[0] for c in range(NCORES)]
    counts = [len(s) for s in sels]
    n_pad = max(128, -(-max(counts) // 128) * 128)

    if n_pad not in _PROGRAM_CACHE:
        _PROGRAM_CACHE[n_pad] = _build_program(n_pad)
    nc = _PROGRAM_CACHE[n_pad]

    kv_flat = kv_pages.reshape(P * S, RD)
    in_maps = []
    for c in range(NCORES):
        sel = sels[c]
        n = counts[c]
        upd = np.empty((n_pad, RD), dtype=out_dtype)
        dest = np.empty((n_pad, 1), dtype=np.int32)
        if n > 0:
            upd[:n] = kv_ev[sel]
            dest[:n, 0] = (tp[sel] - c * PC) * S + ts[sel]
            upd[n:] = upd[n - 1]
            dest[n:, 0] = dest[n - 1, 0]
        else:
            # no updates for this core: rewrite row 0 with its original data
            upd[:] = kv_flat[c * RC]
            dest[:, 0] = 0
        in_maps.append({
            "kv_in": np.ascontiguousarray(kv_flat[c * RC:(c + 1) * RC]),
            "upd": upd,
            "dest": dest,
        })

    global _LAST_IN_MAPS
    _LAST_IN_MAPS = in_maps
    res = run_bass_kernel_spmd(nc, in_maps, core_ids=list(range(NCORES)))
    out = np.concatenate(
        [res.results[c]["kv_out"].reshape(PC, S, HH, D) for c in range(NCORES)],
        axis=0,
    )
    return out.astype(out_dtype, copy=False)


# revision 4
# speedup vs baseline: 16.3929x; 16.3929x over previous
"""KV page-cache scatter update on 8 Trainium2 NeuronCores.

Semantics (matches the reference):
    kv_ev = interleave(new_k, new_v)          # [T, 2H, D], head axis k0,v0,k1,v1,...
    for i in range(K):
        kv_pages[t_pages[i], t_slots[i]] = kv_ev[i]
    return kv_pages

Sharding: kv_pages is split along the page axis across the 8 cores
(256 pages each).  The host partitions the valid tokens by destination
page group and hands each core a compacted, interleaved update block plus
flat destination row indices.  Each core:
  1. copies its 33.5MB page shard input -> output with large DRAM->DRAM DMAs
  2. loads its update rows into SBUF and scatters them into the output with
     an indirect (SWDGE) DMA using the destination row indices.
Destinations are unique (page,slot) pairs, so padding duplicates the last
valid row (identical concurrent writes are benign).
"""

import numpy as np

from concourse import bacc, bass, mybir, tile
from concourse.bass_utils import run_bass_kernel_spmd

# Problem geometry (hardcoded per contract).
P, S, HH, D = 2048, 16, 16, 128   # pages, slots/page, 2*kv_heads, head_dim
T = 2048                          # new tokens
NCORES = 8
PC = P // NCORES                  # pages per core
RC = PC * S                       # flat rows per core (4096)
RD = HH * D                       # row width in f32 (2048 = 8KB)

_PROGRAM_CACHE: dict[int, object] = {}
_LAST_IN_MAPS: list | None = None  # stashed for test.py's traced re-run


def _build_program(n_pad: int):
    """Bass program: copy kv shard in->out, then scatter n_pad update rows."""
    nc = bacc.Bacc("TRN2", target_bir_lowering=False, debug=False)

    kv_in = nc.dram_tensor("kv_in", [RC, RD], mybir.dt.float32, kind="ExternalInput")
    upd = nc.dram_tensor("upd", [n_pad, RD], mybir.dt.float32, kind="ExternalInput")
    dest = nc.dram_tensor("dest", [n_pad, 1], mybir.dt.int32, kind="ExternalInput")
    kv_out = nc.dram_tensor("kv_out", [RC, RD], mybir.dt.float32, kind="ExternalOutput")

    n_blocks = n_pad // 128
    total_elems = RC * RD
    n_chunks = 8
    chunk_elems = total_elems // n_chunks
    # inner descriptor rows of 8192 f32 (32KB), well under the 64KB AP limit
    inner = 8192
    chunk_rows = chunk_elems // inner

    with tile.TileContext(nc) as tc:
        with tc.tile_pool(name="sbuf", bufs=max(2, n_blocks)) as pool:
            # stage all update rows + dest indices into SBUF first; these
            # loads overlap the bulk copy (no dependency on kv_out)
            tiles = []
            for b in range(n_blocks):
                utile = pool.tile([128, RD], mybir.dt.float32)
                dtile = pool.tile([128, 1], mybir.dt.int32)
                nc.scalar.dma_start(out=utile[:], in_=upd[b * 128:(b + 1) * 128, :])
                nc.scalar.dma_start(out=dtile[:], in_=dest[b * 128:(b + 1) * 128, :])
                tiles.append((utile, dtile))

            # bulk copy: 8 chunks x 4MB, DRAM->DRAM, alternating across the
            # two HWDGE rings (SP and ACT) so both descriptor FIFOs stream
            for i in range(n_chunks):
                off = i * chunk_elems
                src = bass.AP(kv_in, off, [[inner, chunk_rows], [1, inner]])
                dst = bass.AP(kv_out, off, [[inner, chunk_rows], [1, inner]])
                eng = nc.sync if i % 2 == 0 else nc.scalar
                eng.dma_start(out=dst, in_=src)

            # scatter update rows into kv_out (serialized after the copy by
            # Tile's WAW tracking on kv_out; destinations are unique rows)
            for utile, dtile in tiles:
                nc.gpsimd.indirect_dma_start(
                    out=kv_out[:],
                    out_offset=bass.IndirectOffsetOnAxis(ap=dtile[:, :1], axis=0),
                    in_=utile[:],
                    in_offset=None,
                )

    nc.compile()
    return nc


def kernel(kv_pages, t_pages, t_slots, new_k, new_v, K):
    kv_pages = np.asarray(kv_pages)
    t_pages = np.asarray(t_pages)
    t_slots = np.asarray(t_slots)
    new_k = np.asarray(new_k)
    new_v = np.asarray(new_v)
    k_valid = int(np.asarray(K))

    out_dtype = kv_pages.dtype
    Tn, Hn, Dn = new_k.shape

    # interleave K/V along the head axis: [T, 2H, D] -> flat [T, RD]
    kv_ev = np.empty((Tn, 2 * Hn, Dn), dtype=out_dtype)
    kv_ev[:, 0::2, :] = new_k
    kv_ev[:, 1::2, :] = new_v
    kv_ev = kv_ev.reshape(Tn, 2 * Hn * Dn)

    tp = t_pages[:k_valid].astype(np.int64)
    ts = t_slots[:k_valid].astype(np.int64)
    core_of = tp // PC

    sels = [np.nonzero(core_of == c)[0] for c in range(NCORES)]
    counts = [len(s) for s in sels]
    n_pad = max(128, -(-max(counts) // 128) * 128)

    if n_pad not in _PROGRAM_CACHE:
        _PROGRAM_CACHE[n_pad] = _build_program(n_pad)
    nc = _PROGRAM_CACHE[n_pad]

    kv_flat = kv_pages.reshape(P * S, RD)
    in_maps = []
    for c in range(NCORES):
        sel = sels[c]
        n = counts[c]
        upd = np.empty((n_pad, RD), dtype=out_dtype)
        dest = np.empty((n_pad, 1), dtype=np.int32)
        if n > 0:
            upd[:n] = kv_ev[sel]
            dest[:n, 0] = (tp[sel] - c * PC) * S + ts[sel]
            upd[n:] = upd[n - 1]
            dest[n:, 0] = dest[n - 1, 0]
        else:
            # no updates for this core: rewrite row 0 with its original data
            upd[:] = kv_flat[c * RC]
            dest[:, 0] = 0
        in_maps.append({
            "kv_in": np.ascontiguousarray(kv_flat[c * RC:(c + 1) * RC]),
            "upd": upd,
            "dest": dest,
        })

    global _LAST_IN_MAPS
    _LAST_IN_MAPS = in_maps
    res = run_bass_kernel_spmd(nc, in_maps, core_ids=list(range(NCORES)))
    out = np.concatenate(
        [res.results[c]["kv_out"].reshape(PC, S, HH, D) for c in range(NCORES)],
        axis=0,
    )
    return out.astype(out_dtype, copy=False)
